# revision 38
# baseline (speedup 1.0000x reference)
"""BEV camera-to-grid scatter-sum kernel for Trainium2 (8 NeuronCores).

Strategy (v3, fp8 + PSUM lane chaining):
  - Host (cheap, O(Np) index math): replicate the reference geometry bit-exactly
    (eager jax on CPU, f32) to get each frustum point's voxel id + kept mask.
  - Kept points (~27%) are sorted by voxel id. The data is heavily clustered
    (~1.3k occupied voxels, ~431 points/voxel), so a BLOCK of 32 consecutive
    128-point tiles usually touches <= 32 distinct voxels. The host greedily
    forms blocks of up to 32 tiles whose voxel-union fits a 32-slot map
    (rare sparse tiles with >32 distinct voxels become rank-windowed chunks).
  - x is quantized to fp8 E3M4 (4 mantissa bits; ~1.3e-2 rel err on the final
    grid vs the 2e-2 gate), halving input DMA - the measured bottleneck.
    Slot codes for ALL blocks ship in one tiny upfront DMA (stored halved:
    e3m4's max finite value is 15.5, so raw indices >= 16 would saturate to
    inf - and inf weights crash the PE). x streams as 2-block pair-DMAs
    alternating between the two HWDGE rings.
  - Device, per block: one-hot S [128, 24] per tile built by is_equal against
    an iota constant on the Vector engine (GpSimd can't run TensorTensor on
    this compiler); the 32 tile matmuls accumulate into 4 PSUM lanes
    (tile u -> column group u%4, fp8 weights, start/stop per lane); 4 blocks
    share one PSUM bank at different free quarters, so PSUM->SBUF f16 lane
    copies (Scalar) and the output DMA (Sync ring) amortize over 4 blocks.
  - Host: add the 4 lane partials, scatter per-block slot sums into the
    [B, NZ*C, NX, NY] grid in float64, unscale, cast to f32; a few hundred
    sparse-tail points (tiles with >SLOTS distinct voxels) are summed on the
    host directly from the exact f32 data.

Blocks are sharded contiguously across the 8 cores; every core runs the
identical NEFF on its own packed slice. Env knobs: BEV_TRACE=1 to capture an
NTFF profile (sets kernel.LAST_EXEC_NS).
"""

import sys
import os
import types
import math

sys.path.insert(0, "/opt/trn_rl_repo")

import numpy as np
import ml_dtypes

# ---- static config (mirrors the nn.Module init_kwargs) ----
IMG_H, IMG_W = 256, 704
FH, FW = 32, 88
D, C = 118, 80
B, N = 1, 6
D0, D1 = 1.0, 60.0
NX, NY, NZ = 360, 360, 1
DXv = np.array([0.3, 0.3, 20.0], np.float32)
BXv = np.array([-54.0 + 0.15, -54.0 + 0.15, 0.0], np.float32)
ALPHA = 1.5

NPTS = B * N * D * FH * FW          # 1,993,728 points
NCORES = 8
SLOTS = 24                          # distinct-voxel slots per block
BT = 32                             # tiles per device block
XSCALE = 2.0                        # fp8 pre-scale (max|2x| ~ 10.8 < 15.5)

LAST_EXEC_NS = None                 # set by kernel() for test harness use


# --------------------------------------------------------------------------
# NTFF profiling hook shim (this image's antenv lacks axon_hooks)
# --------------------------------------------------------------------------
def _install_ntff_hook():
    if "antenv.axon_hooks" in sys.modules:
        return
    mod = types.ModuleType("antenv.axon_hooks")
    mod._hook = None
    mod.set_axon_ntff_profile_hook = lambda h: setattr(mod, "_hook", h)
    mod.get_axon_ntff_profile_hook = lambda: mod._hook
    sys.modules["antenv.axon_hooks"] = mod
    try:
        import antenv
        antenv.axon_hooks = mod
    except ImportError:
        pass
    try:
        from trn_agent_boot.trn_boot import _ntff_profile_via_ctypes
        mod.set_axon_ntff_profile_hook(
            _ntff_profile_via_ctypes("/opt/axon/libaxon_pjrt.so")
        )
    except Exception:
        pass


# --------------------------------------------------------------------------
# Host geometry: bit-exact replica of the reference's index computation
# --------------------------------------------------------------------------
def _host_voxel_ids(camera2lidar, camera_intrinsics, img_aug_matrix,
                    lidar_aug_matrix, denorms):
    """Returns (idx [Np] int64 global voxel ids, kept [Np] bool)."""
    import jax
    import jax.numpy as jnp

    cpu = jax.devices("cpu")[0]

    def geom_fn(sensor2ego, intrin, ida, bda, den):
        Xs, Ys = np.meshgrid(np.linspace(0, IMG_W - 1, FW),
                             np.linspace(0, IMG_H - 1, FH))
        rays = np.stack([Xs, Ys, np.ones_like(Xs), np.ones_like(Xs)], -1)
        rays = jnp.asarray(rays.astype(np.float32))
        d = ((np.arange(D) / D) ** ALPHA).astype(np.float32)
        d = np.broadcast_to(d[:, None, None], (D, FH, FW))
        xg = np.broadcast_to(
            np.linspace(0, IMG_W - 1, FW, dtype=np.float32)[None, None, :],
            (D, FH, FW))
        yg = np.broadcast_to(
            np.linspace(0, IMG_H - 1, FH, dtype=np.float32)[None, :, None],
            (D, FH, FW))
        frustum = np.stack([xg, yg, d, np.ones_like(d)], -1).astype(np.float32)
        frustum = jnp.asarray(frustum)

        ego2sensor = jnp.linalg.inv(sensor2ego)
        O3 = ego2sensor[..., :3, 3]
        n = den[:, :3] / jnp.linalg.norm(den[:, :3], axis=-1, keepdims=True)
        n = n.reshape(B, N, 3)
        nP0 = jnp.sum(n * (O3 + D0 * n), -1)
        nP1 = jnp.sum(n * (O3 + D1 * n), -1)
        Minv = jnp.linalg.inv(intrin) @ jnp.linalg.inv(ida)
        r = jnp.einsum('hwk,bnlk->bnhwl', rays, Minv)[..., :3]
        dirs = r / jnp.linalg.norm(r, axis=-1, keepdims=True)
        ndir = jnp.einsum('bnc,bnhwc->bnhw', n, dirs)
        t0 = nP0[:, :, None, None] / ndir
        tdiff = t0 - nP1[:, :, None, None] / ndir
        z = (t0[:, :, None] - frustum[None, None, ..., 2] * tdiff[:, :, None]) \
            * dirs[..., 2][:, :, None]
        fx = jnp.broadcast_to(frustum[..., 0], (B, N, D, FH, FW))
        fy = jnp.broadcast_to(frustum[..., 1], (B, N, D, FH, FW))
        pts = jnp.stack([fx, fy, z, jnp.ones_like(z)], -1)
        pts = jnp.einsum('bndhwk,bnlk->bndhwl', pts, jnp.linalg.inv(ida))
        pts = jnp.concatenate([pts[..., :2] * pts[..., 2:3], pts[..., 2:]], -1)
        mat = bda[:, None] @ (sensor2ego @ jnp.linalg.inv(intrin))
        geom = jnp.einsum('bndhwk,bnlk->bndhwl', pts, mat)[..., :3]

        g = ((geom.reshape(NPTS, 3) - jnp.asarray(BXv - DXv / 2.0))
             / jnp.asarray(DXv)).astype(jnp.int32)
        kept = ((g[:, 0] >= 0) & (g[:, 0] < NX) & (g[:, 1] >= 0)
                & (g[:, 1] < NY) & (g[:, 2] >= 0) & (g[:, 2] < NZ))
        idx = (g[:, 2] * NX + g[:, 0]) * NY + g[:, 1]
        return idx, kept

    # Run EAGERLY (no jit): XLA fusion perturbs f32 rounding enough to flip
    # a handful of points across voxel boundaries vs the reference's eager
    # op-by-op execution. Bit-exact index agreement matters more than speed.
    with jax.default_device(cpu):
        idx, kept = geom_fn(jnp.asarray(camera2lidar),
                            jnp.asarray(camera_intrinsics),
                            jnp.asarray(img_aug_matrix),
                            jnp.asarray(lidar_aug_matrix),
                            jnp.asarray(denorms))
        idx = np.asarray(idx)
        kept = np.asarray(kept)
    return idx.astype(np.int64), np.asarray(kept)


# --------------------------------------------------------------------------
# Host: greedy block planning over voxel-sorted points
# --------------------------------------------------------------------------
def _plan_blocks(dv, nk, NT):
    """dv: [nk] global distinct-voxel index per sorted point (non-decreasing).
    Returns (blocks, tail_tiles): blocks are (tile_start, ntiles) runs of
    consecutive tiles whose voxel union fits the SLOTS-entry map; the rare
    sparse tiles with >SLOTS distinct voxels (a few hundred points at the
    end of the sorted order) go to tail_tiles for a host-side fallback."""
    blocks = []
    tails = []
    t = 0
    while t < NT:
        p0 = t * 128
        if p0 >= nk:
            break
        d0 = dv[p0]
        g = 0
        while g < BT and t + g < NT:
            pe = min((t + g + 1) * 128, nk) - 1
            if dv[pe] - d0 + 1 <= SLOTS:
                g += 1
            else:
                break
        if g == 0:
            tails.append(t)
            t += 1
        else:
            blocks.append((t, g))
            t += g
    return blocks, tails


# --------------------------------------------------------------------------
# Device kernel (built per nblocks, cached)
# --------------------------------------------------------------------------
_NC_CACHE = {}


def _build_device_kernel(profile):
    """profile: tuple of per-block-slot tile counts (4..BT). Every core runs
    this same NEFF; the host assigns its blocks to slots with enough
    capacity, so pad tiles are never shipped, S-built, or matmul'd."""
    key = tuple(profile)
    if key in _NC_CACHE:
        return _NC_CACHE[key]
    import concourse.bass as bass
    import concourse.tile as tile
    from concourse import bacc, mybir

    f32 = mybir.dt.float32
    f16 = mybir.dt.float16
    fp8 = mybir.dt.float8e3

    nblocks = len(profile)
    xoff = [0]
    for g in profile:
        xoff.append(xoff[-1] + g * C)       # per-slot x offsets (f8 bytes)

    nc = bacc.Bacc("TRN2", target_bir_lowering=False, debug=False)
    OW = C                                  # one [128, C] tile per block
    OGRP = 4                                # blocks per PSUM bank / out DMA
    xpk = nc.dram_tensor("xpk", [128, xoff[-1]], fp8,
                         kind="ExternalInput")
    codes = nc.dram_tensor("codes", [128, nblocks * BT], fp8,
                           kind="ExternalInput")
    iota = nc.dram_tensor("iota", [128, SLOTS], fp8, kind="ExternalInput")
    out = nc.dram_tensor("out", [128, nblocks * OW], f16,
                         kind="ExternalOutput")

    with tile.TileContext(nc) as tc:
        with (
            tc.tile_pool(name="const", bufs=1) as const_pool,
            tc.tile_pool(name="xin", bufs=6) as xin_pool,
            tc.tile_pool(name="smat", bufs=6) as s_pool,
            tc.tile_pool(name="psum", bufs=6, space="PSUM") as psum_pool,
            tc.tile_pool(name="outb", bufs=3) as out_pool,
        ):
            # codes for ALL blocks ride in one tiny upfront DMA so the
            # S-builds never wait on the big x stream (the per-DMA completion
            # semaphore costs ~3us; pay it once, before the pipeline).
            # codes/iota go first on the sync HWDGE ring (SWDGE was measured
            # ~3us slower to signal completion, delaying the first S-build)
            codes_t = const_pool.tile([128, nblocks * BT], fp8)
            nc.sync.dma_start(codes_t[:], codes[:])
            iota_t = const_pool.tile([128, SLOTS], fp8)
            nc.sync.dma_start(iota_t[:], iota[:])

            # x DMA chunks: a single-block first chunk so the first matmuls
            # start ~1us earlier, then pairs alternating between the two
            # HWDGE rings; per-pair granularity keeps the block pipeline
            # fine-grained (4-block chunks measured slower end-to-end)
            chunks = []
            cb0 = 0
            while cb0 < nblocks:
                csz = 1 if cb0 == 0 else min(2, nblocks - cb0)
                chunks.append((cb0, csz))
                cb0 += csz
            chunk_of = {}
            for ci, (cb, csz) in enumerate(chunks):
                for bb in range(cb, cb + csz):
                    chunk_of[bb] = (ci, cb, csz)

            # PE warm-up: the first real matmul lands ~15us into the NEFF,
            # by which time the idle PE is HAM-throttled to 1.2GHz (~3.4us
            # of sustained activity recovers it). Dummy matmuls on zeroed
            # scratch run during the otherwise-idle DMA ramp, targeting
            # group 0's PSUM tile - harmless, since the first real matmul
            # of each lane has start=True and clears has_written.
            scratch = const_pool.tile([128, 112], fp8)
            nc.gpsimd.memset(scratch[:], 0.0)
            ps = psum_pool.tile([128, OGRP * C], f32)
            for _ in range(48):
                nc.tensor.matmul(ps[:SLOTS, :C], scratch[:, :SLOTS],
                                 scratch[:, 32:112], start=True, stop=True,
                                 tile_position=(0, 0))

            xt = None
            ob = None
            for b in range(nblocks):
                g = profile[b]
                ci, cb, csz = chunk_of[b]
                if b == cb:
                    xt = xin_pool.tile([128, xoff[cb + csz] - xoff[cb]], fp8)
                    eng = nc.sync if ci % 2 == 0 else nc.scalar
                    eng.dma_start(xt[:], xpk[:, xoff[cb]:xoff[cb + csz]])
                xq = xoff[b] - xoff[cb]

                st = s_pool.tile([128, BT * SLOTS], fp8)
                # S[p, t*SLOTS + j] = (iota[p, j] == codes[p, t]); all on
                # Vector (GpSimd can't run TensorTensor on this compiler).
                sv = st[:, :g * SLOTS].rearrange("p (t j) -> p t j", j=SLOTS)
                iv = iota_t[:].unsqueeze(1).broadcast_to((128, g, SLOTS))
                cv = codes_t[:, b * BT:b * BT + g].unsqueeze(2) \
                    .broadcast_to((128, g, SLOTS))
                nc.vector.tensor_tensor(sv, iv, cv, mybir.AluOpType.is_equal)

                # g tile-matmuls accumulate into 4 PSUM lanes; 4 blocks
                # share one PSUM bank at different free quarters (group 0
                # reuses the warm-up tile).
                if b % OGRP == 0 and b > 0:
                    ps = psum_pool.tile([128, OGRP * C], f32)
                q = b % OGRP
                for u in range(g):
                    cg = u % 4
                    nc.tensor.matmul(
                        ps[32 * cg:32 * cg + SLOTS, q * C:(q + 1) * C],
                        st[:, u * SLOTS:(u + 1) * SLOTS],
                        xt[:, xq + u * C:xq + (u + 1) * C],
                        start=(u < 4), stop=(u >= g - 4),
                        tile_position=(0, 32 * cg),
                    )

                # once per 4-block group: 4 amortized lane copies (Scalar)
                # + one output DMA on the scalar ring (its queue drains x
                # much earlier than sync's, so the final out isn't stuck
                # behind x traffic); the last group splits copies across
                # Scalar/Vector to shorten the tail
                if q == OGRP - 1 or b == nblocks - 1:
                    g0 = (b // OGRP) * OGRP
                    w = (b + 1 - g0) * C
                    # the last TWO groups split copies across Scalar/Vector:
                    # the Scalar copy backlog was the measured tail critical
                    # path, and Vector is done with S-builds by then
                    late = g0 >= nblocks - 2 * OGRP
                    ob = out_pool.tile([128, OGRP * OW], f16)
                    for k in range(4):
                        dst = ob[32 * k:32 * k + SLOTS, :w]
                        src = ps[32 * k:32 * k + SLOTS, :w]
                        if late and k % 2 == 1:
                            nc.vector.tensor_copy(dst, src)
                        else:
                            nc.scalar.copy(dst, src)
                    nc.scalar.dma_start(out[:, g0 * OW:(b + 1) * OW],
                                        ob[:, :w])

    nc.compile()
    _NC_CACHE[key] = nc
    return nc


# --------------------------------------------------------------------------
# Main entry
# --------------------------------------------------------------------------
def kernel(x, camera2lidar, camera_intrinsics, img_aug_matrix,
           lidar_aug_matrix, denorms):
    global LAST_EXEC_NS
    _install_ntff_hook()
    from concourse import bass_utils

    x = np.asarray(x)
    idx, kept = _host_voxel_ids(camera2lidar, camera_intrinsics,
                                img_aug_matrix, lidar_aug_matrix, denorms)

    # point-level compaction, sorted by voxel id
    keep_pos = np.nonzero(kept)[0]
    keep_pos = keep_pos[np.argsort(idx[keep_pos], kind="stable")]
    nk = len(keep_pos)
    vs = idx[keep_pos]
    dv = np.cumsum(np.r_[True, vs[1:] != vs[:-1]]) - 1  # distinct rank per pt
    ndist = int(dv[-1]) + 1
    first_occ = np.r_[0, np.nonzero(np.diff(dv))[0] + 1]  # rank -> point pos
    NT = max(1, (nk + 127) // 128)

    blocks, tails = _plan_blocks(dv, nk, NT)
    NB = len(blocks)
    per_core = int(math.ceil(NB / NCORES))
    nblocks = per_core

    fp8np = ml_dtypes.float8_e3m4
    # quantize once: [nk] padded to tiles
    x2d = x.reshape(NPTS, C)
    xr = np.zeros((NT * 128, C), dtype=fp8np)
    xr[:nk] = np.clip(x2d[keep_pos] * XSCALE, -15.5, 15.5).astype(fp8np)
    xr = xr.reshape(NT, 128, C)
    dvp = np.full(NT * 128, -(10 ** 9), dtype=np.int64)
    dvp[:nk] = dv

    # codes/iota are stored HALVED: e3m4's max finite value is 15.5, so raw
    # slot indices >= 16 would saturate to inf; c/2 (steps of 0.5 up to 15.5)
    # is exact for all c in [0, 32) and preserves equality.
    iota_np = np.broadcast_to(
        np.arange(SLOTS, dtype=np.float32)[None, :] * 0.5, (128, SLOTS)
    ).astype(fp8np).copy()

    # per-block packed data + slot ids
    blk_ids = []                       # [NB, SLOTS] voxel id per slot (-1 pad)
    xpk_all = np.zeros((NB, 128, BT * C), dtype=fp8np)
    cod_all = np.full((NB, 128, BT), -1.0, dtype=np.float32)
    for i, (t0, g) in enumerate(blocks):
        p0 = t0 * 128
        d0 = int(dv[p0])
        codes = dvp[p0:(t0 + g) * 128] - d0             # [g*128]
        codes = np.where((codes >= 0) & (codes < SLOTS), codes * 0.5,
                         -1.0).astype(np.float32)
        xb = xr[t0:t0 + g]                              # [g, 128, C]
        # layout: [128, BT*C]; tile u's x at free offset u*C
        xpk_all[i, :, :g * C] = xb.transpose(1, 0, 2).reshape(128, g * C)
        cod_all[i, :, :g] = codes.reshape(g, 128).T
        ids = np.full(SLOTS, -1, dtype=np.int64)
        dlast = int(dv[min((t0 + g) * 128, nk) - 1])
        nslot = min(SLOTS, dlast - d0 + 1)
        ranks = d0 + np.arange(nslot)
        ids[:nslot] = vs[first_occ[ranks]]
        blk_ids.append(ids)
    blk_ids = np.array(blk_ids)

    # Stripe blocks across cores by descending tile count so one per-slot
    # capacity profile (baked into the NEFF) fits every core; pad tiles are
    # then never shipped, S-built, or matmul'd.
    g_arr = np.array([g for (_, g) in blocks], dtype=np.int64)
    order = np.argsort(-g_arr, kind="stable")
    assign = np.full((NCORES, nblocks), -1, dtype=np.int64)
    profile = np.full(nblocks, 4, dtype=np.int64)
    # slot order: the SMALLEST stripe goes first (its x lands almost
    # immediately after the stream starts, so the first matmuls clear the
    # ~3.5us DMA-completion-semaphore latency early), then the rest in
    # descending size - which also leaves a small slot at the end for a
    # fast pipeline drain.
    sperm = [nblocks - 1] + list(range(nblocks - 1))
    for j, sj in enumerate(sperm):
        stripe = order[sj * NCORES:(sj + 1) * NCORES]
        assign[:len(stripe), j] = stripe
        if len(stripe):
            profile[j] = max(4, int(g_arr[stripe[0]]))
    xoff = np.zeros(nblocks + 1, dtype=np.int64)
    xoff[1:] = np.cumsum(profile * C)

    in_maps = []
    core_ids_list = []
    for k in range(NCORES):
        xp = np.zeros((128, int(xoff[-1])), dtype=fp8np)
        cp = np.full((nblocks, 128, BT), -1.0, dtype=np.float32)
        for j in range(nblocks):
            bid = assign[k, j]
            if bid < 0:
                continue
            g = int(g_arr[bid])
            xp[:, xoff[j]:xoff[j] + g * C] = xpk_all[bid][:, :g * C]
            cp[j] = cod_all[bid]
        in_maps.append({
            "xpk": np.ascontiguousarray(xp),
            "codes": np.ascontiguousarray(
                cp.astype(fp8np).transpose(1, 0, 2)
                .reshape(128, nblocks * BT)),
            "iota": iota_np,
        })
        core_ids_list.append(k)

    nc = _build_device_kernel(tuple(int(g) for g in profile))
    res = bass_utils.run_bass_kernel_spmd(
        nc, in_maps, core_ids=core_ids_list,
        trace=bool(int(os.environ.get("BEV_TRACE", "0"))),
    )
    LAST_EXEC_NS = res.exec_time_ns

    # host combine (float64 accumulate): sum 4 lanes, scatter slot sums
    G = np.zeros((B * NZ * NX * NY, C), dtype=np.float64)
    for k in range(NCORES):
        jsel = np.nonzero(assign[k] >= 0)[0]
        if len(jsel) == 0:
            continue
        od = res.results[k]["out"]                  # [128, nblocks*C]
        o = od.reshape(4, 32, nblocks, C)[:, :SLOTS].astype(np.float64)
        o = o.sum(axis=0).transpose(1, 0, 2)        # [nblocks, SLOTS, C]
        o = o[jsel]
        ids = blk_ids[assign[k, jsel]]
        valid = ids >= 0
        np.add.at(G, ids[valid], o[valid])

    G /= XSCALE
    # host fallback for the sparse tail (a few hundred points whose tiles
    # exceed the SLOTS-entry map) - exact f32 data, no quantization
    for t in tails:
        p0, p1 = t * 128, min((t + 1) * 128, nk)
        np.add.at(G, vs[p0:p1], x2d[keep_pos[p0:p1]].astype(np.float64))
    out = G.astype(np.float32).reshape(B, NZ, NX, NY, C)
    return np.ascontiguousarray(
        out.transpose(0, 1, 4, 2, 3).reshape(B, NZ * C, NX, NY)
    )
